# revision 45
# baseline (speedup 1.0000x reference)
"""MoE routing kernel (nn_DecFCSwitch) for 8 Trainium2 NeuronCores.

Reference computes all 16 expert branches for every token and then
selects one per token.  Only the selected branch matters, so:

  host:   sort tokens by expert; the first 256 tokens of each expert go
          to the device (capacity CAP=256), the rare overflow tokens are
          computed exactly on the host in fp32.  relu(x), the residual
          add and the output bias also stay on host.  Device operands
          are quantized to fp8 with power-of-two scales and packed into
          one per-core blob laid out in exact consumption order.
  device: expert-parallel SPMD — core i owns experts {2i, 2i+1} and runs
          a 2-layer MLP on its experts' tokens.  Both layers run DoubleRow
          fp8e4 matmuls (each instruction contracts a 256-row pair in one
          pass, 0.5 cycles/column); the ACT/DVE engines requantize the
          hidden to fp8e4 (scale folded into the affine).  Pairs of
          output d-tiles share one PSUM bank so layer-2 evictions are
          single [128, 512] scaled copies to fp8e4.
  host:   transpose back, un-scale, add x + b_out, scatter to token
          order; merge the overflow rows.

Scales (powers of two, exact): h*32, W_in*4096, hid*64, W_out*2048 (all
e4m3).  Layer-1 evict: hid8 = fp8e4(psum/2048 + 64*b_in).  Layer-2
evict: y8 = fp8e4(psum/2048); host computes x + y8/64 + b_out.  Measured
rel err ~1.34e-2 vs the 2e-2 gate (the residual x dominates the output
norm, damping the quantization error ~4x).
"""

import os
import sys

import numpy as np

for _p in ("/opt/trn_rl_repo", "/root/.axon_site/_ro/trn_rl_repo"):
    if os.path.isdir(_p) and _p not in sys.path:
        sys.path.insert(0, _p)

import ml_dtypes

B, D, S, NB = 4096, 1024, 256, 16
NCORES = 8
EPC = NB // NCORES  # experts per core
KD = D // 128  # d-dim k/m tiles
KS = S // 128  # s-dim tiles
DQ = D // 4  # d-quarter (w2 block width in elements)
CAP = 256  # device tokens per expert; the rest overflow to the host

E3 = ml_dtypes.float8_e3m4
E4 = ml_dtypes.float8_e4m3
BF16 = ml_dtypes.bfloat16

SH = 32.0  # h scale (e4m3, max 240)
SW1 = 4096.0  # W_in scale (e4m3, max 240)
SH2 = 64.0  # hidden scale (e4m3 — DoubleRow needs fp8e4/e5)
SW2 = 2048.0  # W_out scale (e4m3, max 240)
# layer-1 ACT: out = psum * (SH2/(SH*SW1)) + SH2*b_in
S1 = SH2 / (SH * SW1)
# layer-2: y = psum ; host computes x + y/SY + b_out
SY = SH2 * SW2

_programs = {}  # C -> compiled Bacc program
LAST_RESULT = None  # BassKernelResults of the most recent run (for test.py)

N_WARM = 56  # PE warm-up matmuls (HAM p-state ramp) before real data lands


# Blob column layout (fp8 bytes per partition), in consumption order:
#   for k in 0..7: [ w1e0_k (S) | h_k (EPC*C) ]
#   then w1e1 (KD*S), w2e0 (4*KS*DQ), w2e1 (4*KS*DQ)
def _blob_cols(C):
    blk = S + EPC * C
    base_w1e1 = KD * blk
    base_w2 = [base_w1e1 + KD * S, base_w1e1 + KD * S + 4 * KS * DQ]
    total = base_w2[1] + 4 * KS * DQ
    return blk, base_w1e1, base_w2, total


def default_cfg(C):
    blk, bw1e1, bw2, total = _blob_cols(C)
    return {
        "n_warm": N_WARM,
        # loads: (col0, col1, eng) into the blob, program order
        "cuts": [
            (0, 4 * blk, "sync"),
            (4 * blk, 8 * blk, "sync"),
            (bw2[0], bw2[0] + 1024, "sync"),
            (bw1e1, bw2[0], "sync"),
            (bw2[0] + 1024, bw2[1], "sync"),
            (bw2[1], bw2[1] + 1024, "sync"),
            (bw2[1] + 1024, total, "sync"),
        ],
        "bias_eng": "gpsimd",
        # stores: (e, pair) -> list of (k0, k1, eng); pair mp covers
        # d-tiles 2mp and 2mp+1
        "stores": {
            (0, 1): [(0, 4, "sync")],
            (0, 3): [(4, 8, "gpsimd")],
            (1, 1): [(0, 4, "sync")],
            (1, 2): [(4, 6, "gpsimd")],
            (1, 3): [(6, 8, "sync")],
        },
        # eviction engine per (e, pair): alternate ACT / DVE
        "evict_eng": {
            (e, mp): ("act" if (e * 4 + mp) % 2 == 0 else "dve")
            for e in range(EPC)
            for mp in range(4)
        },
        "l1_4way": False,
    }


def _build_program(C, cfg=None):
    cfg = {**default_cfg(C), **(cfg or {})}
    import concourse.mybir as mybir
    import concourse.tile as tile
    from concourse import bacc

    f8 = mybir.dt.float8e3
    f84 = mybir.dt.float8e4
    bf = mybir.dt.bfloat16
    f32 = mybir.dt.float32
    ident = mybir.ActivationFunctionType.Identity
    DR = mybir.MatmulPerfMode.DoubleRow

    blk, bw1e1, bw2, FW = _blob_cols(C)

    nc = bacc.Bacc()
    xb = nc.declare_dram_parameter("xb", [128, FW], f8, isOutput=False)
    # bc[:, e*KS + t] = SH2*b_in tile t of expert e
    bc = nc.declare_dram_parameter("bc", [128, EPC * KS], f32, isOutput=False)
    # y stored as fp8e4 (psum * 2^-11), per-partition contiguous k-runs so
    # store descriptors stay >= 512B
    yT = nc.declare_dram_parameter("yT", [128, EPC, KD * C], f84, isOutput=True)

    with tile.TileContext(nc) as tc:
        with (
            tc.tile_pool(name="bias", bufs=1) as bias_pool,
            tc.tile_pool(name="xs", bufs=1) as x_pool,
            tc.tile_pool(name="hid", bufs=2) as hid_pool,
            tc.tile_pool(name="yout", bufs=2) as y_pool,
            tc.tile_pool(name="ps1", bufs=2, space="PSUM") as ps1_pool,
            tc.tile_pool(
                name="ps2", bufs=cfg.get("ps2_bufs", 4), space="PSUM"
            ) as ps2_pool,
            tc.tile_pool(name="warm", bufs=1) as warm_pool,
        ):
            engines = {"sync": nc.sync, "gpsimd": nc.gpsimd, "scalar": nc.scalar}

            # Dummy matmuls keep the PE busy from t=0 so the HAM throttle is
            # fully ramped by the time the first real operands arrive.
            wz = warm_pool.tile([128, 64], f8, tag="wz")
            nc.gpsimd.memset(wz[:], 0)
            wact = warm_pool.tile([128, 1], bf, tag="wact")
            # Dummy activation fires the ACT table load (1283 ns) during the
            # DMA lead-in instead of on the first hid eviction.
            nc.scalar.activation(wact[:], wz[:, 0:1], ident)
            if cfg.get("warm_dve", False):
                wdve = warm_pool.tile([128, 1], bf, tag="wdve")
                nc.vector.tensor_scalar_add(wdve[:], wz[:, 0:1], 0.0)
            wps = ps2_pool.tile([128, 2 * C], f32, name="wps", tag="ps")
            for _ in range(cfg["n_warm"]):
                nc.tensor.matmul(
                    wps[0:64, 0:64], lhsT=wz[:, 0:64], rhs=wz[:], start=True, stop=True
                )

            bct = bias_pool.tile([128, EPC * KS], f32, tag="bc")
            engines[cfg["bias_eng"]].dma_start(out=bct[:], in_=bc[:, :])

            def b1_ap(e, t):
                return bct[:, e * KS + t : e * KS + t + 1]

            # One SBUF blob mirrors the dram blob; loads are contiguous
            # column ranges, so every DMA moves maximal-size descriptors and
            # chunk boundaries are schedule knobs, not layout constraints.
            xt = x_pool.tile([128, FW], f8, tag="xt")

            def w1_slice(e, k, t):
                base = k * blk if e == 0 else bw1e1 + k * S
                return xt[:, base + t * 128 : base + t * 128 + 128]

            def h_slice(k, e):  # rhs [128, C] for d-tile k, expert e
                base = k * blk + S + e * C
                return xt[:, base : base + C]

            def w1_slice_dr(e, kp, t):
                # DoubleRow lhsT [128, 2, 128]: dim1 spans d-tiles 2kp,2kp+1
                if e == 0:
                    return (
                        xt[:, 2 * kp * blk : (2 * kp + 2) * blk]
                        .bitcast(f84)
                        .rearrange("p (j b) -> p j b", j=2)[
                            :, :, t * 128 : t * 128 + 128
                        ]
                    )
                base = bw1e1 + 2 * kp * S
                return (
                    xt[:, base : base + 2 * S]
                    .bitcast(f84)
                    .rearrange("p (j s) -> p j s", j=2)[:, :, t * 128 : t * 128 + 128]
                )

            def h_slice_dr(kp, e):  # rhs [128, 2, C] for d-tile pair kp
                return (
                    xt[:, 2 * kp * blk : (2 * kp + 2) * blk]
                    .bitcast(f84)
                    .rearrange("p (j b) -> p j b", j=2)[
                        :, :, S + e * C : S + e * C + C
                    ]
                )

            def w2_slice_dr(e, m):
                # DoubleRow lhsT [128, 2, 128]: dim1 spans the two s-tiles
                # (stride DQ in the (q, t, dq) blob layout), viewed as e4m3.
                q, r = divmod(m * 128, DQ)
                base = bw2[e] + q * (KS * DQ)
                return (
                    xt[:, base : base + KS * DQ]
                    .bitcast(f84)
                    .rearrange("p (t d) -> p t d", t=2)[:, :, r : r + 128]
                )

            for c0, c1, eng in cfg["cuts"]:
                engines[eng].dma_start(out=xt[:, c0:c1], in_=xb[:, c0:c1])

            # ---- Layer 1 (k-interleaved so each arriving chunk feeds the
            # PE immediately), then Layer 2 --------------------------------
            all_hids = []
            NKP = KD // 2
            experts_k = (
                [(e, kp) for kp in range(NKP) for e in range(EPC)]
                if cfg["l1_4way"]
                else [(e, kp) for e in range(EPC) for kp in range(NKP)]
            )
            pss = {
                e: ps1_pool.tile([128, 2 * C], f32, name=f"ps1_{e}", tag="ps1")
                for e in range(EPC)
            }
            for e, kp in experts_k:
                for t in range(KS):
                    # Only the bank's FIRST matmul sets start: a later
                    # start=True in the same PSUM bank re-marks the whole
                    # 2KB zero-region, wiping the sibling half's first
                    # accumulation term (verified on HW).  The t=1 half
                    # reads as zero on its first accumulate via the same
                    # region-zero.
                    nc.tensor.matmul(
                        pss[e][:, t * C : (t + 1) * C],
                        lhsT=w1_slice_dr(e, kp, t),
                        rhs=h_slice_dr(kp, e),
                        perf_mode=DR,
                        start=(kp == 0 and t == 0),
                        stop=(kp == NKP - 1),
                        skip_group_check=True,
                    )
                if kp == NKP - 1:
                    hid_pair = hid_pool.tile(
                        [128, 2 * C], f84, name=f"hid_{e}", tag="hid"
                    )
                    # t0 on ACT, t1 on DVE so the two requants run in
                    # parallel and L2 unblocks sooner.
                    nc.scalar.activation(
                        hid_pair[:, 0:C],
                        pss[e][:, 0:C],
                        ident,
                        bias=b1_ap(e, 0),
                        scale=S1,
                    )
                    hd = cfg.get("hid_dve", True)
                    if hd == "e0" and e == 1:
                        hd = False
                    if hd:
                        nc.vector.tensor_scalar(
                            hid_pair[:, C : 2 * C],
                            pss[e][:, C : 2 * C],
                            S1,
                            b1_ap(e, 1),
                            op0=mybir.AluOpType.mult,
                            op1=mybir.AluOpType.add,
                        )
                    else:
                        nc.scalar.activation(
                            hid_pair[:, C : 2 * C],
                            pss[e][:, C : 2 * C],
                            ident,
                            bias=b1_ap(e, 1),
                            scale=S1,
                        )
                    all_hids.append((e, hid_pair))

            all_hids.sort(key=lambda x: x[0])

            for e in range(EPC):
                hid_pair = all_hids[e][1]
                hid3 = hid_pair[:].rearrange("p (t c) -> p t c", t=2)
                # Layer 2: y^T[d, c] = sum_s W_out[d, s] * hid^T[s, c].
                # One DoubleRow matmul per d-tile; d-tile pairs share a PSUM
                # bank so the eviction is a [128, 2C] pure copy to bf16.
                y_big = y_pool.tile([128, KD * C], f84, name=f"y_big_{e}", tag="y")
                for mp in range(KD // 2):
                    ps = ps2_pool.tile(
                        [128, 2 * C], f32, name=f"ps2_{e}_{mp}", tag="ps"
                    )
                    for half in range(2):
                        nc.tensor.matmul(
                            ps[:, half * C : (half + 1) * C],
                            lhsT=w2_slice_dr(e, 2 * mp + half),
                            rhs=hid3,
                            perf_mode=DR,
                            start=True,
                            stop=True,
                        )
                    dst = y_big[:, 2 * mp * C : (2 * mp + 2) * C]
                    # evict = psum * 2^-11 quantized to fp8e4 (host divides
                    # by SY/2048 and adds b_out)
                    if cfg["evict_eng"][(e, mp)] == "act":
                        nc.scalar.activation(dst, ps[:], ident, scale=2.0**-11)
                    else:
                        nc.vector.tensor_scalar_mul(dst, ps[:], 2.0**-11)
                    for k0, k1, issuer in cfg["stores"].get((e, mp), ()):
                        engines[issuer].dma_start(
                            out=yT[:, e, k0 * C : k1 * C],
                            in_=y_big[:, k0 * C : k1 * C],
                        )

    nc.compile()
    return nc


def kernel(x, y_index, W_in, b_in, W_out, b_out):
    global LAST_RESULT
    from concourse.bass_utils import run_bass_kernel_spmd

    x = np.asarray(x, dtype=np.float32)
    W_in = np.asarray(W_in, dtype=np.float32)
    b_in = np.asarray(b_in, dtype=np.float32)
    W_out = np.asarray(W_out, dtype=np.float32)
    b_out = np.asarray(b_out, dtype=np.float32)
    eidx = np.asarray(y_index).reshape(-1).astype(np.int64)

    C = CAP
    counts = np.bincount(eidx, minlength=NB)

    # --- host dispatch: group tokens by expert; overflow beyond CAP is
    # computed exactly on the host ---------------------------------------
    order = np.argsort(eidx, kind="stable")
    starts = np.zeros(NB + 1, dtype=np.int64)
    np.cumsum(counts, out=starts[1:])

    h = np.maximum(x, 0.0)
    Xg = np.zeros((NB, C, D), dtype=np.float32)
    dev_toks = []
    ovf_toks = []
    for e in range(NB):
        toks = order[starts[e] : starts[e + 1]]
        n = min(len(toks), C)
        Xg[e, :n] = h[toks[:n]]
        dev_toks.append(toks[:n])
        ovf_toks.append(toks[n:])

    # Quantize to fp8 with power-of-two scales, then pack everything the
    # device streams into one per-core blob [128, FW] laid out in exact
    # consumption order (see _blob_cols).
    hp_all = (
        (Xg * SH)
        .astype(E4)
        .reshape(NCORES, EPC * C, KD, 128)
        .transpose(0, 2, 3, 1)
    )  # [NCORES, KD, 128, EPC*C]
    w1_all = (
        (W_in * SW1)
        .astype(E4)
        .reshape(NCORES, EPC, S, KD, 128)
        .transpose(0, 1, 4, 3, 2)
    )  # [NCORES, EPC, 128, KD, S]
    w2_all = (
        (W_out * SW2)
        .astype(E4)
        .reshape(NCORES, EPC, 4, DQ, KS, 128)
        .transpose(0, 1, 5, 2, 4, 3)
        .reshape(NCORES, EPC, 128, 4 * KS * DQ)
    )
    blk, bw1e1, bw2, FW = _blob_cols(C)
    xb_all = np.empty((NCORES, 128, FW), dtype=E3)
    for k in range(KD):
        xb_all[:, :, k * blk : k * blk + S] = w1_all[:, 0, :, k].view(E3)
        xb_all[:, :, k * blk + S : (k + 1) * blk] = hp_all[:, k].view(E3)
    xb_all[:, :, bw1e1 : bw1e1 + KD * S] = (
        w1_all[:, 1].reshape(NCORES, 128, KD * S).view(E3)
    )
    xb_all[:, :, bw2[0] : bw2[0] + 4 * KS * DQ] = w2_all[:, 0].view(E3)
    xb_all[:, :, bw2[1] : bw2[1] + 4 * KS * DQ] = w2_all[:, 1].view(E3)

    # [NCORES, 128, EPC*KS] contiguous per partition
    bc_all = np.ascontiguousarray(
        (SH2 * b_in).reshape(NCORES, EPC * KS, 128).transpose(0, 2, 1)
    )

    if C not in _programs:
        _programs[C] = _build_program(C)
    nc = _programs[C]

    in_maps = [
        {
            "xb": np.ascontiguousarray(xb_all[i]),
            "bc": np.ascontiguousarray(bc_all[i]),
        }
        for i in range(NCORES)
    ]

    trace = bool(int(os.environ.get("KERNEL_TRACE", "0")))
    res = run_bass_kernel_spmd(nc, in_maps, list(range(NCORES)), trace=trace)
    LAST_RESULT = res

    # --- host gather: transpose back, un-scale, add x + b_out, scatter ---
    out = np.empty_like(x)
    # yT [128, EPC, KD*C] fp8e4 holding sel * (SY/2048); -> [NB, C, D]
    Yg = np.stack(
        [
            np.asarray(r["yT"])
            .view(E4)
            .astype(np.float32)
            .reshape(128, EPC, KD, C)
            .transpose(1, 2, 0, 3)
            .reshape(EPC, D, C)
            for r in res.results
        ]
    )  # [NCORES, EPC, D, C]
    Yg = Yg.transpose(0, 1, 3, 2).reshape(NB, C, D) * (2048.0 / SY)
    for e in range(NB):
        toks = dev_toks[e]
        out[toks] = x[toks] + Yg[e, : len(toks)] + b_out[e]
        ovf = ovf_toks[e]
        if len(ovf):
            hid = h[ovf] @ W_in[e].T + b_in[e]
            out[ovf] = x[ovf] + hid @ W_out[e].T + b_out[e]
    return out
